# revision 3
# baseline (speedup 1.0000x reference)
"""Trainium2 Bass kernel for nn_CombinedGraphReadout (pooling, 8 NeuronCores).

Strategy
--------
Shard by graph: core c owns graphs [1024c, 1024(c+1)). Each 128-graph tile's
nodes are padded to a uniform budget L128 (multiple of 128), so one SPMD
program with a fully static schedule serves all 8 cores. Node phase iterates
per graph-tile over its L128-node span (512-wide segments for matmuls).

All matmul operands bf16, fp32 PSUM accumulation:
  - h1T = relu(W1^T @ xT + b1)  (bias+relu fused in the PSUM evacuation,
    alternating ACT/DVE 2:1 to keep the PE fed)
  - scores natural via lhsT=h1T slices (bias via K=1 ones-row matmul),
    exp on ACT (sigmoid = reciprocal(1+exp(-x)) to stay on one ACT table)
  - values natural * ex (stride-0 free-dim broadcast) = wv on DVE; per
    segment all values matmuls are emitted before the wv multiplies and
    U/S accumulations so DVE latency hides under PE work
  - segment sums U|S = onehot^T @ [wv | ex], one-hots precomputed on host
    and DMA-streamed; PSUM-accumulated across a graph-tile's sub-tiles
  - segment max: tensor_tensor_scan with additive -1e30 reset mask
    (pre-replicated on host, DMA-streamed), ap_gather at per-graph
    last-node positions
  - graph level: mean = U/S + bt2, sum = U + S*bt2, PE transpose, then
    Wc/Wmax/Wfinal matmuls with relu fused in evacuations
Host: gather/one-hot prep, empty-graph row zeroing, final transpose.
"""

import numpy as np

import concourse.bacc as bacc
import concourse.tile as tile
from concourse import mybir

F32 = mybir.dt.float32
BF16 = mybir.dt.bfloat16
I16 = mybir.dt.int16
I32 = mybir.dt.int32

NODE_DIM = 256
HID = 512
OUT_DIM = 512
N_NODES = 131072
N_GRAPHS = 8192
N_CORES = 8
GC = N_GRAPHS // N_CORES          # 1024 graphs per core
NGT = GC // 128                   # 8 graph-tiles per core
NEG = -1.0e30


def _bf16(a):
    import ml_dtypes
    return np.ascontiguousarray(np.asarray(a, dtype=np.float32)
                                .astype(ml_dtypes.bfloat16))


_PROG_CACHE = {}


def build_program(L128, loop_n=1):
    """L128 = padded nodes per graph-tile (multiple of 128). loop_n > 1
    repeats the computation on-device (timing only)."""
    key = (L128, loop_n)
    if key in _PROG_CACHE:
        return _PROG_CACHE[key]
    assert L128 % 128 == 0

    W = NGT * L128
    SPG = L128 // 128              # sub-tiles per graph-tile
    SEG = (L128 + 511) // 512      # 512-wide segments per graph-tile

    nc = bacc.Bacc("TRN2", target_bir_lowering=False)

    xT = nc.dram_tensor("xT", [NGT, 2, 128, L128], BF16, kind="ExternalInput")
    maskd = nc.dram_tensor("maskd", [NGT, 128, L128], BF16, kind="ExternalInput")
    oh = nc.dram_tensor("oh", [NGT, 128, SPG * 128], BF16, kind="ExternalInput")
    gidx = nc.dram_tensor("gidx", [NGT, 128, 8], I16, kind="ExternalInput")
    w1 = nc.dram_tensor("w1", [4, 2, 128, HID], BF16, kind="ExternalInput")
    b1 = nc.dram_tensor("b1", [4, 128, 4], F32, kind="ExternalInput")
    ws2 = nc.dram_tensor("ws2", [2, 4, 128, 16], BF16, kind="ExternalInput")
    bs2r = nc.dram_tensor("bs2r", [2, 1, 16], BF16, kind="ExternalInput")
    ones1 = nc.dram_tensor("ones1", [1, 128], BF16, kind="ExternalInput")
    wt2 = nc.dram_tensor("wt2", [2, 4, 128, HID], BF16, kind="ExternalInput")
    bt2rep = nc.dram_tensor("bt2rep", [2, 128, HID], BF16, kind="ExternalInput")
    wc = nc.dram_tensor("wc", [2, 4, 128, OUT_DIM], BF16, kind="ExternalInput")
    wmax = nc.dram_tensor("wmax", [2, 128, OUT_DIM], BF16, kind="ExternalInput")
    wfinal = nc.dram_tensor("wfinal", [12, 128, OUT_DIM], BF16, kind="ExternalInput")
    OUT = nc.dram_tensor("outT", [4, 128, GC], F32, kind="ExternalOutput")

    Relu = mybir.ActivationFunctionType.Relu
    Exp = mybir.ActivationFunctionType.Exp
    Copy = mybir.ActivationFunctionType.Copy
    AL = mybir.AluOpType

    from contextlib import ExitStack
    with tile.TileContext(nc) as tc, ExitStack() as ctx:
        if True:
            ctx.enter_context(nc.allow_low_precision(
                reason="bf16 intermediates; matmul/segment accumulation in fp32 PSUM"))
            wgt = ctx.enter_context(tc.tile_pool(name="wgt", bufs=1))
            acc = ctx.enter_context(tc.tile_pool(name="acc", bufs=1))
            xtp = ctx.enter_context(tc.tile_pool(name="xt", bufs=2))
            mkp = ctx.enter_context(tc.tile_pool(name="mk", bufs=2))
            scp = ctx.enter_context(tc.tile_pool(name="scan", bufs=1))
            gip = ctx.enter_context(tc.tile_pool(name="gi", bufs=2))
            ohp = ctx.enter_context(tc.tile_pool(name="ohp", bufs=2))
            h1p = ctx.enter_context(tc.tile_pool(name="h1", bufs=2))
            exp_ = ctx.enter_context(tc.tile_pool(name="ex", bufs=2))
            wvp = ctx.enter_context(tc.tile_pool(name="wv", bufs=1))
            pbp = ctx.enter_context(tc.tile_pool(name="pb", bufs=1))
            pbs = ctx.enter_context(tc.tile_pool(name="pbs", bufs=2))
            psp = ctx.enter_context(tc.tile_pool(name="ps", bufs=4, space="PSUM"))
            psu0 = ctx.enter_context(tc.tile_pool(name="psu0", bufs=1, space="PSUM"))
            psu1 = ctx.enter_context(tc.tile_pool(name="psu1", bufs=1, space="PSUM"))
            pss0 = ctx.enter_context(tc.tile_pool(name="pss0", bufs=1, space="PSUM"))
            pss1 = ctx.enter_context(tc.tile_pool(name="pss1", bufs=1, space="PSUM"))
            upool = (psu0, psu1)
            spool = (pss0, pss1)

            # ---- resident weights (phase-A weights first: they gate tile 0)
            w1_t = [[wgt.tile([128, HID], BF16, tag=f"w1_{m}_{k}", name=f"w1_{m}_{k}")
                     for k in range(2)] for m in range(4)]
            b1_t = [wgt.tile([128, 4], F32, tag=f"b1_{m}", name=f"b1_{m}")
                    for m in range(4)]
            for m in range(4):
                nc.gpsimd.dma_start(b1_t[m][:], b1[m])
                for k in range(2):
                    nc.gpsimd.dma_start(w1_t[m][k][:], w1[m, k])
            ws2_t = [[wgt.tile([128, 16], BF16, tag=f"ws2_{b}_{k}", name=f"ws2_{b}_{k}")
                      for k in range(4)] for b in range(2)]
            bs2_t = [wgt.tile([1, 16], BF16, tag=f"bs2_{b}", name=f"bs2_{b}")
                     for b in range(2)]
            wt2_t = [[wgt.tile([128, HID], BF16, tag=f"wt2_{b}_{k}", name=f"wt2_{b}_{k}")
                      for k in range(4)] for b in range(2)]
            bt2_t = [wgt.tile([128, HID], BF16, tag=f"bt2_{b}", name=f"bt2_{b}")
                     for b in range(2)]
            for b in range(2):
                nc.gpsimd.dma_start(bs2_t[b][:], bs2r[b])
                nc.gpsimd.dma_start(bt2_t[b][:], bt2rep[b])
                for k in range(4):
                    nc.gpsimd.dma_start(ws2_t[b][k][:], ws2[b, k])
                    nc.gpsimd.dma_start(wt2_t[b][k][:], wt2[b, k])
            ones_t = wgt.tile([1, 128], BF16, tag="ones1", name="ones1")
            nc.gpsimd.dma_start(ones_t[:], ones1[:])

            from concourse.masks import make_identity
            ident = wgt.tile([128, 128], F32, tag="ident", name="ident")
            make_identity(nc, ident[:])
            zeros1 = wgt.tile([128, 1], F32, tag="zeros1", name="zeros1")
            nc.vector.memset(zeros1[:], 0.0)

            # phase-B weights queued after everything phase A needs
            wc_t = [[wgt.tile([128, OUT_DIM], BF16, tag=f"wc_{b}_{k}",
                              name=f"wc_{b}_{k}") for k in range(4)]
                    for b in range(2)]
            wmax_t = [wgt.tile([128, OUT_DIM], BF16, tag=f"wmax_{k}",
                               name=f"wmax_{k}") for k in range(2)]
            wf_t = [wgt.tile([128, OUT_DIM], BF16, tag=f"wf_{k}", name=f"wf_{k}")
                    for k in range(12)]
            for b in range(2):
                for k in range(4):
                    nc.gpsimd.dma_start(wc_t[b][k][:], wc[b, k])
            for k in range(2):
                nc.gpsimd.dma_start(wmax_t[k][:], wmax[k])
            for k in range(12):
                nc.gpsimd.dma_start(wf_t[k][:], wfinal[k])

            # ---- accumulators ----
            maxT = [acc.tile([128, GC], F32, tag=f"maxT_{i}", name=f"maxT_{i}")
                    for i in range(2)]
            U_sb = [[acc.tile([128, HID], BF16, tag=f"U_{b}_{g}", name=f"U_{b}_{g}")
                     for g in range(NGT)] for b in range(2)]
            S_sb = [[acc.tile([128, 16], F32, tag=f"S_{b}_{g}", name=f"S_{b}_{g}")
                     for g in range(NGT)] for b in range(2)]

            evac_i = [0]

            def evac_relu_bias(out_ap, in_ap, bias_ap, fd):
                # spread PSUM evacuations ACT:DVE = 2:1 (Pool lacks the ISA)
                if evac_i[0] % 3 == 2:
                    nc.vector.scalar_tensor_tensor(
                        out=out_ap, in0=in_ap, scalar=bias_ap,
                        in1=zeros1[:].to_broadcast([128, fd]),
                        op0=AL.add, op1=AL.max)
                else:
                    nc.scalar.activation(out=out_ap, in_=in_ap, func=Relu,
                                         bias=bias_ap, scale=1.0)
                evac_i[0] += 1

            u_ps = [None, None]
            s_ps = [None, None]

            loop_ctx = tc.For_i(0, loop_n, 1) if loop_n > 1 else None
            if loop_ctx is not None:
                loop_ctx.__enter__()

            # ================= node phase: per graph-tile =================
            for gt in range(NGT):
                xsp = [xtp.tile([128, L128], BF16, tag=f"xsp{i}", name=f"xsp{i}")
                       for i in range(2)]
                nc.sync.dma_start(xsp[0][:], xT[gt, 0])
                nc.sync.dma_start(xsp[1][:], xT[gt, 1])
                mask_sb = mkp.tile([128, L128], BF16, tag="mask", name="mask")
                nc.sync.dma_start(mask_sb[:], maskd[gt])
                oh_sb = ohp.tile([128, SPG * 128], BF16, tag="oh", name="oh")
                nc.sync.dma_start(oh_sb[:], oh[gt])
                gi = gip.tile([128, 8], I16, tag="gix", name="gix")
                nc.sync.dma_start(gi[:], gidx[gt])
                sc_span = [scp.tile([128, L128], F32, tag=f"scan{i}", name=f"scan{i}")
                           for i in range(2)]

                for s in range(SEG):
                    c0 = s * 512
                    cw = min(512, L128 - c0)
                    nsub = cw // 128
                    seg = slice(c0, c0 + cw)

                    for i in range(2):
                        init = NEG if s == 0 else sc_span[i][:, c0 - 1:c0]
                        nc.vector.tensor_tensor_scan(
                            out=sc_span[i][:, seg], data0=mask_sb[:, seg],
                            data1=xsp[i][:, seg],
                            initial=init, op0=AL.add, op1=AL.max)

                    # one-hot blocks for this segment's sub-tiles (host-built)
                    ohs = [oh_sb[:, (s * 4 + j) * 128:(s * 4 + j + 1) * 128]
                           for j in range(nsub)]

                    for b in range(2):
                        h1s, h1t = [], []
                        for role, dst in ((0, h1s), (1, h1t)):
                            m = 2 * b + role
                            for mt in range(4):
                                ps = psp.tile([128, 512], F32, tag="ps", name="ps")
                                for kt in range(2):
                                    nc.tensor.matmul(
                                        out=ps[:, :cw],
                                        lhsT=w1_t[m][kt][:, mt * 128:(mt + 1) * 128],
                                        rhs=xsp[kt][:, seg],
                                        start=(kt == 0), stop=(kt == 1))
                                h = h1p.tile([128, 512], BF16, tag=f"h1_{role}_{mt}",
                                             name=f"h1_{role}_{mt}")
                                evac_relu_bias(h[:, :cw], ps[:, :cw],
                                               b1_t[m][:, mt:mt + 1], cw)
                                dst.append(h)

                        ps_sc = psp.tile([128, 512], F32, tag="ps", name="ps")
                        for j in range(nsub):
                            cs = slice(16 * j, 16 * j + 16)
                            rs = slice(128 * j, 128 * j + 128)
                            for kt in range(4):
                                nc.tensor.matmul(
                                    out=ps_sc[:, cs], lhsT=h1s[kt][:, rs],
                                    rhs=ws2_t[b][kt][:],
                                    start=(kt == 0), stop=False)
                            nc.tensor.matmul(out=ps_sc[:, cs], lhsT=ones_t[:],
                                             rhs=bs2_t[b][:], start=False, stop=True)
                        exw = 16 * nsub
                        ex = exp_.tile([128, 64], BF16, tag=f"ex{b}", name=f"ex{b}")
                        if b == 0:
                            nc.scalar.activation(out=ex[:, :exw], in_=ps_sc[:, :exw],
                                                 func=Exp, scale=1.0)
                        else:
                            ef = exp_.tile([128, 64], F32, tag="ef", name="ef")
                            nc.scalar.activation(out=ef[:, :exw], in_=ps_sc[:, :exw],
                                                 func=Exp, scale=-1.0)
                            nc.vector.tensor_scalar(out=ef[:, :exw], in0=ef[:, :exw],
                                                    scalar1=1.0, scalar2=None,
                                                    op0=AL.add)
                            nc.vector.reciprocal(out=ex[:, :exw], in_=ef[:, :exw])

                        # all values matmuls first, then the wv multiplies,
                        # then the U/S accumulations: DVE latency hides under
                        # PE work instead of stalling it per sub-tile
                        ps_vs = []
                        for j in range(nsub):
                            ps_v = psp.tile([128, 512], F32, tag="ps", name="ps")
                            for kt in range(4):
                                nc.tensor.matmul(
                                    out=ps_v[:], lhsT=h1t[kt][:, 128 * j:128 * j + 128],
                                    rhs=wt2_t[b][kt][:],
                                    start=(kt == 0), stop=(kt == 3))
                            ps_vs.append(ps_v)
                        wvs = []
                        for j in range(nsub):
                            cs = slice(16 * j, 16 * j + 16)
                            wv = wvp.tile([128, HID], BF16, tag=f"wv{b}_{j}",
                                          name=f"wv{b}_{j}")
                            nc.vector.tensor_tensor(
                                out=wv[:].rearrange("p (h c) -> p h c", h=16),
                                in0=ps_vs[j][:].rearrange("p (h c) -> p h c", h=16),
                                in1=ex[:, cs].to_broadcast([128, 16, 32]),
                                op=AL.mult)
                            wvs.append(wv)
                        for j in range(nsub):
                            st = s * 4 + j
                            cs = slice(16 * j, 16 * j + 16)
                            first = (st == 0)
                            last = (st == SPG - 1)
                            if first:
                                u_ps[b] = upool[b].tile([128, HID], F32,
                                                        tag=f"u{b}", name=f"u{b}")
                                s_ps[b] = spool[b].tile([128, 16], F32,
                                                        tag=f"s{b}", name=f"s{b}")
                            nc.tensor.matmul(out=u_ps[b][:], lhsT=ohs[j],
                                             rhs=wvs[j][:], start=first, stop=last)
                            nc.tensor.matmul(out=s_ps[b][:], lhsT=ohs[j],
                                             rhs=ex[:, cs], start=first, stop=last)
                            if last:
                                if b == 0:
                                    nc.scalar.activation(out=U_sb[b][gt][:],
                                                         in_=u_ps[b][:], func=Copy,
                                                         scale=1.0)
                                else:
                                    nc.vector.tensor_copy(out=U_sb[b][gt][:],
                                                          in_=u_ps[b][:])
                                nc.vector.tensor_scalar(out=S_sb[b][gt][:],
                                                        in0=s_ps[b][:],
                                                        scalar1=1e-30,
                                                        scalar2=None, op0=AL.add)

                for i in range(2):
                    nc.gpsimd.ap_gather(
                        out_ap=maxT[i][:, 128 * gt:128 * gt + 128].unsqueeze(2),
                        in_ap=sc_span[i][:].unsqueeze(2),
                        idxs_ap=gi[:], channels=128, num_elems=L128, d=1,
                        num_idxs=128)

            # ================= graph phase =================
            maxTb = [pbp.tile([128, GC], BF16, tag=f"maxTb_{i}", name=f"maxTb_{i}")
                     for i in range(2)]
            for i in range(2):
                nc.vector.tensor_copy(out=maxTb[i][:], in_=maxT[i][:])

            pgT = [[pbp.tile([128, GC], BF16, tag=f"pgT_{b}_{kt}",
                             name=f"pgT_{b}_{kt}") for kt in range(4)]
                   for b in range(2)]
            for b in range(2):
                for gt in range(NGT):
                    scr = pbs.tile([128, HID], F32, tag="scr", name="scr")
                    pg = pbs.tile([128, HID], F32, tag="pg", name="pg")
                    Uv = U_sb[b][gt][:].rearrange("p (h c) -> p h c", h=16)
                    if b == 0:
                        rS = pbs.tile([128, 16], F32, tag="rS", name="rS")
                        nc.vector.reciprocal(out=rS[:], in_=S_sb[b][gt][:])
                        nc.vector.tensor_tensor(
                            out=scr[:].rearrange("p (h c) -> p h c", h=16),
                            in0=Uv, in1=rS[:].to_broadcast([128, 16, 32]),
                            op=AL.mult)
                        nc.vector.tensor_tensor(out=pg[:], in0=scr[:],
                                                in1=bt2_t[b][:], op=AL.add)
                    else:
                        nc.vector.tensor_tensor(
                            out=scr[:].rearrange("p (h c) -> p h c", h=16),
                            in0=S_sb[b][gt][:].to_broadcast([128, 16, 32]),
                            in1=bt2_t[b][:].rearrange("p (h c) -> p h c", h=16),
                            op=AL.mult)
                        nc.vector.tensor_tensor(out=pg[:], in0=U_sb[b][gt][:],
                                                in1=scr[:], op=AL.add)
                    for blk in range(4):
                        tps = psp.tile([128, 512], F32, tag="ps", name="ps")
                        nc.tensor.transpose(out=tps[:, :128],
                                            in_=pg[:, 128 * blk:128 * blk + 128],
                                            identity=ident[:])
                        dst = pgT[b][blk][:, 128 * gt:128 * gt + 128]
                        if (gt + blk) % 2 == 0:
                            nc.scalar.activation(out=dst, in_=tps[:, :128],
                                                 func=Copy, scale=1.0)
                        else:
                            nc.vector.tensor_copy(out=dst, in_=tps[:, :128])

            rawT = [pbp.tile([128, GC], BF16, tag=f"rawT_{i}", name=f"rawT_{i}")
                    for i in range(12)]
            for b in range(2):
                for mt in range(4):
                    for hf in range(2):
                        hs = slice(512 * hf, 512 * hf + 512)
                        ps = psp.tile([128, 512], F32, tag="ps", name="ps")
                        for kt in range(4):
                            nc.tensor.matmul(
                                out=ps[:],
                                lhsT=wc_t[b][kt][:, 128 * mt:128 * mt + 128],
                                rhs=pgT[b][kt][:, hs],
                                start=(kt == 0), stop=(kt == 3))
                        dst = rawT[4 * b + mt][:, hs]
                        if (mt + hf) % 2 == 0:
                            nc.scalar.activation(out=dst, in_=ps[:], func=Relu,
                                                 scale=1.0)
                        else:
                            nc.vector.tensor_scalar(out=dst, in0=ps[:], scalar1=0.0,
                                                    scalar2=None, op0=AL.max)
            for mt in range(4):
                for hf in range(2):
                    hs = slice(512 * hf, 512 * hf + 512)
                    ps = psp.tile([128, 512], F32, tag="ps", name="ps")
                    for kt in range(2):
                        nc.tensor.matmul(
                            out=ps[:], lhsT=wmax_t[kt][:, 128 * mt:128 * mt + 128],
                            rhs=maxTb[kt][:, hs], start=(kt == 0), stop=(kt == 1))
                    dst = rawT[8 + mt][:, hs]
                    if (mt + hf) % 2 == 0:
                        nc.scalar.activation(out=dst, in_=ps[:], func=Relu, scale=1.0)
                    else:
                        nc.vector.tensor_scalar(out=dst, in0=ps[:], scalar1=0.0,
                                                scalar2=None, op0=AL.max)

            for mt in range(4):
                ot = pbs.tile([128, GC], F32, tag="outT", name="outT")
                for hf in range(2):
                    hs = slice(512 * hf, 512 * hf + 512)
                    ps = psp.tile([128, 512], F32, tag="ps", name="ps")
                    for kt in range(12):
                        nc.tensor.matmul(
                            out=ps[:], lhsT=wf_t[kt][:, 128 * mt:128 * mt + 128],
                            rhs=rawT[kt][:, hs], start=(kt == 0), stop=(kt == 11))
                    if hf % 2 == 0:
                        nc.scalar.activation(out=ot[:, hs], in_=ps[:],
                                             func=Copy, scale=1.0)
                    else:
                        nc.vector.tensor_copy(out=ot[:, hs], in_=ps[:])
                nc.sync.dma_start(OUT[mt], ot[:])

            if loop_ctx is not None:
                loop_ctx.__exit__(None, None, None)

    nc.compile()
    _PROG_CACHE[key] = nc
    return nc


# ---------------------------------------------------------------------------
# host-side preparation
# ---------------------------------------------------------------------------

def prepare(node_embeddings, node_to_graph_id,
            mWs1, mbs1, mWs2, mbs2, mWt1, mbt1, mWt2, mbt2, mWc,
            sWs1, sbs1, sWs2, sbs2, sWt1, sbt1, sWt2, sbt2, sWc,
            Wmax, Wfinal):
    """Returns (L128, in_maps, empty_mask)."""
    x = np.asarray(node_embeddings, dtype=np.float32)
    gid = np.asarray(node_to_graph_id, dtype=np.int64)
    assert x.shape == (N_NODES, NODE_DIM)

    bnd = np.searchsorted(gid, np.arange(N_GRAPHS + 1))
    cnt = np.diff(bnd)
    empty_mask = cnt == 0

    gt_cnt = cnt.reshape(N_GRAPHS // 128, 128).sum(axis=1)
    L128 = int(np.ceil(gt_cnt.max() / 128) * 128)
    W = NGT * L128
    SPG = L128 // 128

    def lhsT_tiles(Wm, nkt, ncol):
        Wm = np.asarray(Wm, np.float32)
        assert Wm.shape == (nkt * 128, ncol)
        return _bf16(Wm.reshape(nkt, 128, ncol))

    w1_a = np.stack([lhsT_tiles(w, 2, HID) for w in (mWs1, mWt1, sWs1, sWt1)])
    b1_a = np.stack([np.asarray(b, np.float32).reshape(4, 128).T
                     for b in (mbs1, mbt1, sbs1, sbt1)])
    ws2_a = np.stack([lhsT_tiles(w, 4, 16) for w in (mWs2, sWs2)])
    bs2_a = np.stack([_bf16(np.asarray(b, np.float32).reshape(1, 16))
                      for b in (mbs2, sbs2)])
    ones_a = _bf16(np.ones((1, 128), np.float32))
    wt2_a = np.stack([lhsT_tiles(w, 4, HID) for w in (mWt2, sWt2)])
    bt2_a = np.stack([_bf16(np.tile(np.asarray(b, np.float32)[None, :], (128, 1)))
                      for b in (mbt2, sbt2)])
    wc_a = np.stack([lhsT_tiles(w, 4, OUT_DIM) for w in (mWc, sWc)])
    wmax_a = lhsT_tiles(Wmax, 2, OUT_DIM)
    wf_a = lhsT_tiles(Wfinal, 12, OUT_DIM)

    in_maps = []
    for c in range(N_CORES):
        g0 = GC * c
        xs = np.zeros((W, NODE_DIM), np.float32)
        lgrel = np.full(W, -1, np.int32)
        reset = np.zeros(W, bool)
        last_pos = np.zeros(GC, np.int64)
        prev_last = 0
        for t in range(NGT):
            a, e = bnd[g0 + 128 * t], bnd[g0 + 128 * (t + 1)]
            n = e - a
            base = t * L128
            xs[base:base + n] = x[a:e]
            gl = (gid[a:e] - (g0 + 128 * t)).astype(np.int32)
            lgrel[base:base + n] = gl
            reset[base] = True
            if n > 1:
                reset[base + 1:base + n][gl[1:] != gl[:-1]] = True
            if n < L128:
                reset[base + n] = True
            ends = np.searchsorted(gid[a:e], np.arange(128 * t, 128 * (t + 1))
                                   + g0 + 1) - 1
            for j in range(128):
                g = 128 * t + j
                if cnt[g0 + g] > 0:
                    prev_last = base + ends[j]
                last_pos[g] = prev_last
        reset[0] = True

        # [NGT, 2, 128, L128]
        xT_a = _bf16(xs.T.reshape(2, 128, NGT, L128).transpose(2, 0, 1, 3))
        mask_a = _bf16(np.broadcast_to(
            np.where(reset, NEG, 0.0).astype(np.float32)
            .reshape(NGT, 1, L128), (NGT, 128, L128)))
        ids = lgrel.reshape(NGT, SPG, 128)
        oh_a = _bf16((ids[:, :, :, None] == np.arange(128)[None, None, None, :])
                     .transpose(0, 2, 1, 3).reshape(NGT, 128, SPG * 128))

        gidx_a = np.zeros((NGT, 128, 8), np.int16)
        for t in range(NGT):
            loc = (last_pos[128 * t:128 * (t + 1)] - t * L128)
            loc = np.clip(loc, 0, L128 - 1).astype(np.int16)
            wrapped = loc.reshape(8, 16).T
            gidx_a[t] = np.tile(wrapped, (8, 1)).reshape(128, 8)

        in_maps.append({
            "xT": xT_a, "maskd": mask_a, "oh": oh_a, "gidx": gidx_a,
            "w1": w1_a, "b1": b1_a, "ws2": ws2_a, "bs2r": bs2_a,
            "ones1": ones_a, "wt2": wt2_a, "bt2rep": bt2_a, "wc": wc_a,
            "wmax": wmax_a, "wfinal": wf_a,
        })
    return L128, in_maps, empty_mask


def kernel(node_embeddings, node_to_graph_id, num_graphs, **weights):
    from concourse.bass_utils import run_bass_kernel_spmd

    L128, in_maps, empty_mask = prepare(node_embeddings, node_to_graph_id,
                                        **weights)
    nc = build_program(L128)
    res = run_bass_kernel_spmd(nc, in_maps, core_ids=list(range(N_CORES)))
    out = np.empty((N_GRAPHS, OUT_DIM), np.float32)
    for c in range(N_CORES):
        oT = res.results[c]["outT"]
        out[GC * c:GC * (c + 1)] = oT.reshape(OUT_DIM, GC).T
    out[empty_mask] = 0.0
    return out

